# revision 4
# baseline (speedup 1.0000x reference)
"""Trainium2 Bass kernel for PrivateGraphSAGE (2-layer PrivSAGEConv).

Push-mode distribution (8 NeuronCores, SPMD):
  - Nodes (x, noise, output) sharded across cores (6250 rows each).
  - Edges partitioned by SOURCE owner: each core computes partial messages
    msg_c[dst] = sum over its own-shard sources, for ALL destinations, using
    only its local clipped table (no AllGather!).
  - Per 128-dst tile: dma_gather pulls source rows from the core's own
    bf16 table; a one-hot built on DVE is the stationary matmul operand so
    the TensorEngine scatters the segment-sum into PSUM; PSUM partial tiles
    are staged wide and DMA'd to a [50176, 128] bf16 partial table.
  - A ReduceScatter (add) sums the 8 partial tables and leaves each core
    its own destination shard of the messages (output is only N/8 per core,
    which is far cheaper than an AllGather of the full table).
  - Epilogue per own tile: agg = xc + msg + noise; PE transpose + matmul
    with W.T; layer 1 fuses SELU + the next layer's clip and writes the
    layer-2 gather table; layer 2 writes the output.
"""

import contextlib

import numpy as np

import concourse.bacc as bacc
import concourse.bass as bass
import concourse.mybir as mybir
import concourse.tile as tile
from concourse.bass_utils import run_bass_kernel_spmd

F32 = mybir.dt.float32
BF16 = mybir.dt.bfloat16
I16 = mybir.dt.int16

SELU_LAM = 1.0507009873554804934193349852946
SELU_ALPHA = 1.6732632423543772848170429916717

N_NODES = 50000
NCORES = 8


# ---------------------------------------------------------------------------
# Host-side preprocessing
# ---------------------------------------------------------------------------

def _preprocess(src, dst, n_nodes=N_NODES, ncores=NCORES):
    """Partition edges by source owner; bucket by global 128-dst tile; pad
    each bucket to G[tile]*128 edges with G uniform across cores (SPMD)."""
    S = -(-n_nodes // ncores)          # 6250 rows per shard
    NT = -(-S // 128)                  # 49 tiles per shard
    SPAD = NT * 128                    # 6272
    GT = ncores * NT                   # 392 global dst tiles

    src = np.asarray(src, np.int64)
    dst = np.asarray(dst, np.int64)
    c = src // S                       # owner core (src < 50000 -> c <= 7)
    lsrc = src - c * S                 # local row in owner's table
    cd = dst // S
    ld = dst - cd * S
    gt = cd * NT + ld // 128           # global padded dst tile
    rel = ld % 128

    key = c * GT + gt
    counts = np.bincount(key, minlength=ncores * GT).reshape(ncores, GT)
    G = np.maximum(1, -(-counts.max(axis=0) // 128))       # [GT]
    off = np.concatenate([[0], np.cumsum(G)[:-1]]).astype(np.int64)
    g_tot = int(G.sum())
    e_pad = g_tot * 128

    order = np.argsort(key, kind="stable")
    key_s = key[order]
    lsrc_s = lsrc[order]
    rel_s = rel[order]
    run_start = np.concatenate(
        [[0], np.cumsum(np.bincount(key_s, minlength=ncores * GT))[:-1]])
    within = np.arange(len(key_s)) - run_start[key_s]
    slot = off[key_s % GT] * 128 + within
    cc = key_s // GT

    idxp = np.zeros((ncores, e_pad), np.int16)
    tagp = np.full((ncores, e_pad), -1.0, np.float32)
    idxp[cc, slot] = lsrc_s.astype(np.int16)
    tagp[cc, slot] = rel_s

    # int16 gather indices: idx j of the flat edge array lives at
    # [j % 16, j // 16]; replicated across all eight 16-row bands because
    # different Q7 ucode versions read different bands.
    idx16 = idxp.reshape(ncores, e_pad // 16, 16).transpose(0, 2, 1)
    idx16 = np.ascontiguousarray(np.tile(idx16, (1, 8, 1)))

    # dst tags: [128 edge slots, g_tot groups]
    drel = np.ascontiguousarray(
        tagp.reshape(ncores, g_tot, 128).transpose(0, 2, 1))

    meta = dict(n_nodes=n_nodes, ncores=ncores, S=S, NT=NT, SPAD=SPAD,
                GT=GT, G=G, off=off, g_tot=g_tot)
    return meta, idx16, drel


# ---------------------------------------------------------------------------
# Device program
# ---------------------------------------------------------------------------

def _build_onehot(nc, oh, drel_sb, c, gs, iota_sb):
    """oh[e, g*128 + d] = (dstrel[e, c+g] == d), one DVE op."""
    d3 = drel_sb[:, c:c + gs].to_broadcast([128, gs, 128])
    ii = iota_sb[:]
    i3 = bass.AP(ii.tensor, ii.offset, [list(ii.ap[0]), [0, gs], list(ii.ap[1])])
    o3 = oh[:, :gs * 128].rearrange("p (g e) -> p g e", e=128)
    nc.vector.tensor_tensor(o3, d3, i3, op=mybir.AluOpType.is_equal)


def _wide_dram(t, r0, nrows):
    """DRAM rows [r0, r0+nrows*128) as [128, nrows, 128]."""
    return t[r0 * 128:(r0 + nrows) * 128, :].rearrange("(g p) f -> p g f", p=128)


def _wide_sbuf(t, nrows):
    return t[:, :nrows * 128].rearrange("p (g f) -> p g f", f=128)


def _build_program(meta, with_b):
    m = meta
    S, NT, SPAD, GT = m["S"], m["NT"], m["SPAD"], m["GT"]
    G, off, g_tot = m["G"], m["off"], m["g_tot"]
    ncores = m["ncores"]
    NTAB = ncores * SPAD
    Gmax = int(G.max())
    rg = [list(range(ncores))]

    nc = bacc.Bacc(None, target_bir_lowering=False)

    xs = nc.declare_dram_parameter("xs", [SPAD, 128], F32, isOutput=False)
    n1s = nc.declare_dram_parameter("n1s", [SPAD, 128], F32, isOutput=False)
    n2s = nc.declare_dram_parameter("n2s", [SPAD, 128], F32, isOutput=False)
    w1t = nc.declare_dram_parameter("w1t", [128, 128], F32, isOutput=False)
    w2t = nc.declare_dram_parameter("w2t", [128, 128], F32, isOutput=False)
    idxp = nc.declare_dram_parameter("idx", [128, g_tot * 8], I16, isOutput=False)
    drel = nc.declare_dram_parameter("dstrel", [128, g_tot], F32, isOutput=False)
    iotap = nc.declare_dram_parameter("iota", [128, 128], F32, isOutput=False)
    identp = nc.declare_dram_parameter("ident", [128, 128], F32, isOutput=False)
    if with_b:
        b1p = nc.declare_dram_parameter("b1r", [1, 128], F32, isOutput=False)
        b2p = nc.declare_dram_parameter("b2r", [1, 128], F32, isOutput=False)
    outp = nc.declare_dram_parameter("out", [SPAD, 128], F32, isOutput=True)

    xtab = nc.dram_tensor("xtab", [SPAD, 128], BF16)
    htab = nc.dram_tensor("htab", [SPAD, 128], BF16)
    msgp1 = nc.dram_tensor("msgp1", [NTAB, 128], BF16)
    msgp2 = nc.dram_tensor("msgp2", [NTAB, 128], BF16)
    msgs1 = nc.dram_tensor("msgs1", [SPAD, 128], BF16)
    msgs2 = nc.dram_tensor("msgs2", [SPAD, 128], BF16)

    mult = mybir.AluOpType.mult
    add = mybir.AluOpType.add
    Act = mybir.ActivationFunctionType

    from concourse.library_config import mlp
    nc.gpsimd.load_library(mlp)

    with tile.TileContext(nc) as tc:
        with contextlib.ExitStack() as ctx:
            cpool = ctx.enter_context(tc.tile_pool(name="const", bufs=1))
            xin = ctx.enter_context(tc.tile_pool(name="xin", bufs=3))
            pa = ctx.enter_context(tc.tile_pool(name="pa", bufs=4))
            pa1 = ctx.enter_context(tc.tile_pool(name="pa1", bufs=4))
            selfp = ctx.enter_context(tc.tile_pool(name="selfp", bufs=1))
            stgb = ctx.enter_context(tc.tile_pool(name="stgb", bufs=3))
            stgp = ctx.enter_context(tc.tile_pool(name="stgp", bufs=3))
            gp = ctx.enter_context(tc.tile_pool(name="gather", bufs=8))
            ohp = ctx.enter_context(tc.tile_pool(name="onehot", bufs=4))
            mrp = ctx.enter_context(tc.tile_pool(name="mread", bufs=2))
            nrp = ctx.enter_context(tc.tile_pool(name="nread", bufs=2))
            ep = ctx.enter_context(tc.tile_pool(name="epil", bufs=4))
            eps = ctx.enter_context(tc.tile_pool(name="epilsc", bufs=4))
            psA = ctx.enter_context(tc.tile_pool(name="psA", bufs=4, space="PSUM"))
            psT = ctx.enter_context(tc.tile_pool(name="psT", bufs=2, space="PSUM"))
            psO = ctx.enter_context(tc.tile_pool(name="psO", bufs=2, space="PSUM"))

            # ---- constants ------------------------------------------------
            w1t_sb = cpool.tile([128, 128], F32, tag="w1t")
            nc.sync.dma_start(w1t_sb[:], w1t[:])
            w2t_sb = cpool.tile([128, 128], F32, tag="w2t")
            nc.sync.dma_start(w2t_sb[:], w2t[:])
            iota_sb = cpool.tile([128, 128], F32, tag="iota")
            nc.sync.dma_start(iota_sb[:], iotap[:])
            ident_sb = cpool.tile([128, 128], F32, tag="ident")
            nc.sync.dma_start(ident_sb[:], identp[:])
            idx_sb = cpool.tile([128, g_tot * 8], I16, tag="idx")
            nc.sync.dma_start(idx_sb[:], idxp[:])
            drel_sb = cpool.tile([128, g_tot], F32, tag="drel")
            nc.sync.dma_start(drel_sb[:], drel[:])
            if with_b:
                b1_sb = cpool.tile([1, 128], F32, tag="b1")
                nc.sync.dma_start(b1_sb[:], b1p[:])
                b2_sb = cpool.tile([1, 128], F32, tag="b2")
                nc.sync.dma_start(b2_sb[:], b2p[:])
                ones_sb = cpool.tile([1, 128], F32, tag="ones")
                nc.gpsimd.memset(ones_sb[:], 1.0)
            lnal_sb = cpool.tile([128, 1], F32, tag="lnal")
            nc.gpsimd.memset(lnal_sb[:], float(np.log(SELU_ALPHA)))

            xcs = [None] * NT     # persistent f32 clipped x tiles
            hcs = [None] * NT     # persistent f32 clipped h tiles

            # ---- phase A: clip own x shard --------------------------------
            stw = None
            ws = 0
            for t in range(NT):
                if t % 4 == 0:
                    nb = min(4, NT - t)
                    xw = xin.tile([128, 512], F32, tag="xw")
                    nc.sync.dma_start(_wide_sbuf(xw, nb), _wide_dram(xs, t, nb))
                xt = xw[:, (t % 4) * 128:(t % 4 + 1) * 128]
                sq = pa.tile([128, 128], F32, tag="sq")
                ss = pa1.tile([128, 1], F32, tag="ss")
                nc.scalar.activation(sq[:], xt, Act.Square, accum_out=ss[:])
                nrm = pa1.tile([128, 1], F32, tag="nrm")
                nc.scalar.activation(nrm[:], ss[:], Act.Sqrt)
                dd = pa1.tile([128, 1], F32, tag="dd")
                nc.vector.tensor_scalar_max(dd[:], nrm[:], 1.0)
                sc = pa1.tile([128, 1], F32, tag="sc")
                nc.vector.reciprocal(sc[:], dd[:])
                xc_t = selfp.tile([128, 128], F32, tag=f"xc{t}")
                nc.vector.tensor_tensor(xc_t[:], xt, sc[:].to_broadcast([128, 128]),
                                        op=mult)
                xcs[t] = xc_t
                if t % 8 == 0:
                    stw = stgb.tile([128, 1024], BF16, tag="stx")
                    ws = t
                nc.scalar.activation(stw[:, (t % 8) * 128:(t % 8 + 1) * 128],
                                     xc_t[:], Act.Copy)
                if t % 8 == 7 or t == NT - 1:
                    nb = t - ws + 1
                    nc.sync.dma_start(_wide_dram(xtab, ws, nb), _wide_sbuf(stw, nb))

            # ---- edge phase ----------------------------------------------
            def edge_phase(tab, msgp, lname):
                ncalls = (g_tot + 7) // 8
                gts = []
                for k in range(ncalls):
                    ng = min(8, g_tot - k * 8)
                    gtile = gp.tile([128, 1024], BF16, tag=f"g{lname}")
                    nc.gpsimd.dma_gather(
                        gtile[:, :ng * 128].rearrange("p (g e) -> p g e", e=128),
                        tab[:, :],
                        idx_sb[:, k * 64:k * 64 + ng * 8],
                        ng * 128, ng * 128, 128)
                    gts.append(gtile)
                stw = None
                ws = 0
                for t in range(GT):
                    Gt = int(G[t])
                    ot = int(off[t])
                    oh = ohp.tile([128, Gmax * 128], BF16, tag="oh")
                    _build_onehot(nc, oh, drel_sb, ot, Gt, iota_sb)
                    pag = psA.tile([128, 128], F32, tag="pag")
                    for j in range(Gt):
                        k, s = divmod(ot + j, 8)
                        nc.tensor.matmul(
                            pag[:],
                            lhsT=oh[:, j * 128:(j + 1) * 128],
                            rhs=gts[k][:, s * 128:(s + 1) * 128],
                            start=(j == 0), stop=(j == Gt - 1))
                    if t % 8 == 0:
                        stw = stgp.tile([128, 1024], BF16, tag=f"stp{lname}")
                        ws = t
                    dsl = stw[:, (t % 8) * 128:(t % 8 + 1) * 128]
                    if t % 2 == 0:
                        nc.vector.tensor_copy(dsl, pag[:])
                    else:
                        nc.scalar.activation(dsl, pag[:], Act.Copy)
                    if t % 8 == 7 or t == GT - 1:
                        nb = t - ws + 1
                        nc.sync.dma_start(_wide_dram(msgp, ws, nb),
                                          _wide_sbuf(stw, nb))

            # ---- epilogue --------------------------------------------------
            def epilogue(msgs, noise, wt_sb, b_sb, self_tiles, out_tiles,
                         selu, lname):
                mw = nw = None
                for t in range(NT):
                    if t % 8 == 0:
                        nb = min(8, NT - t)
                        mw = mrp.tile([128, 1024], BF16, tag=f"mw{lname}")
                        nc.sync.dma_start(_wide_sbuf(mw, nb),
                                          _wide_dram(msgs, t, nb))
                    if t % 4 == 0:
                        nb = min(4, NT - t)
                        nw = nrp.tile([128, 512], F32, tag=f"nw{lname}")
                        nc.sync.dma_start(_wide_sbuf(nw, nb),
                                          _wide_dram(noise, t, nb))
                    a1 = ep.tile([128, 128], F32, tag="a1")
                    nc.vector.tensor_tensor(
                        a1[:], self_tiles[t][:],
                        mw[:, (t % 8) * 128:(t % 8 + 1) * 128], op=add)
                    agg = ep.tile([128, 128], F32, tag="agg")
                    nc.vector.tensor_tensor(
                        agg[:], a1[:],
                        nw[:, (t % 4) * 128:(t % 4 + 1) * 128], op=add)
                    pt = psT.tile([128, 128], F32, tag="pt")
                    nc.tensor.transpose(pt[:], agg[:], ident_sb[:])
                    agT = ep.tile([128, 128], F32, tag="agT")
                    if t % 2 == 0:
                        nc.vector.tensor_copy(agT[:], pt[:])
                    else:
                        nc.scalar.activation(agT[:], pt[:], Act.Copy)
                    po = psO.tile([128, 128], F32, tag="po")
                    if b_sb is not None:
                        nc.tensor.matmul(po[:], lhsT=ones_sb[:], rhs=b_sb[:],
                                         start=True, stop=False)
                        nc.tensor.matmul(po[:], lhsT=agT[:], rhs=wt_sb[:],
                                         start=False, stop=True)
                    else:
                        nc.tensor.matmul(po[:], lhsT=agT[:], rhs=wt_sb[:],
                                         start=True, stop=True)
                    if t % 8 == 0:
                        ostw = (stgb if selu else stgp).tile(
                            [128, 1024], BF16 if selu else F32,
                            tag=f"so{lname}")
                        ws = t
                    if selu:
                        t0 = ep.tile([128, 128], F32, tag="t0")
                        nc.vector.tensor_scalar_min(t0[:], po[:], 0.0)
                        e_ = ep.tile([128, 128], F32, tag="e_")
                        nc.scalar.activation(e_[:], t0[:], Act.Exp,
                                             bias=lnal_sb[:])
                        m_ = ep.tile([128, 128], F32, tag="m_")
                        nc.vector.tensor_scalar_max(m_[:], po[:], 0.0)
                        u_ = ep.tile([128, 128], F32, tag="u_")
                        nc.vector.tensor_tensor(u_[:], m_[:], e_[:], op=add)
                        hh = ep.tile([128, 128], F32, tag="hh")
                        nc.scalar.activation(hh[:], u_[:], Act.Copy,
                                             bias=-SELU_LAM * SELU_ALPHA,
                                             scale=SELU_LAM)
                        sq2 = ep.tile([128, 128], F32, tag="sq2")
                        ss2 = eps.tile([128, 1], F32, tag="ss2")
                        nc.scalar.activation(sq2[:], hh[:], Act.Square,
                                             accum_out=ss2[:])
                        nr2 = eps.tile([128, 1], F32, tag="nr2")
                        nc.scalar.activation(nr2[:], ss2[:], Act.Sqrt)
                        dd2 = eps.tile([128, 1], F32, tag="dd2")
                        nc.vector.tensor_scalar_max(dd2[:], nr2[:], 1.0)
                        sc2 = eps.tile([128, 1], F32, tag="sc2")
                        nc.vector.reciprocal(sc2[:], dd2[:])
                        hc_t = selfp.tile([128, 128], F32, tag=f"hc{t}")
                        nc.vector.tensor_tensor(
                            hc_t[:], hh[:], sc2[:].to_broadcast([128, 128]),
                            op=mult)
                        out_tiles[t] = hc_t
                        nc.scalar.activation(
                            ostw[:, (t % 8) * 128:(t % 8 + 1) * 128],
                            hc_t[:], Act.Copy)
                        if t % 8 == 7 or t == NT - 1:
                            nb = t - ws + 1
                            nc.sync.dma_start(_wide_dram(htab, ws, nb),
                                              _wide_sbuf(ostw, nb))
                    else:
                        nc.scalar.activation(
                            ostw[:, (t % 8) * 128:(t % 8 + 1) * 128],
                            po[:], Act.Copy)
                        if t % 8 == 7 or t == NT - 1:
                            nb = t - ws + 1
                            nc.sync.dma_start(_wide_dram(outp, ws, nb),
                                              _wide_sbuf(ostw, nb))

            # ---- layer 1 ---------------------------------------------------
            edge_phase(xtab, msgp1, "a")
            nc.gpsimd.collective_compute(
                "ReduceScatter", add, ins=[msgp1[:, :]], outs=[msgs1[:, :]],
                replica_groups=rg)
            epilogue(msgs1, n1s, w1t_sb, b1_sb if with_b else None,
                     xcs, hcs, selu=True, lname="a")

            # ---- layer 2 ---------------------------------------------------
            edge_phase(htab, msgp2, "b")
            nc.gpsimd.collective_compute(
                "ReduceScatter", add, ins=[msgp2[:, :]], outs=[msgs2[:, :]],
                replica_groups=rg)
            epilogue(msgs2, n2s, w2t_sb, b2_sb if with_b else None,
                     hcs, [None] * NT, selu=False, lname="b")

    nc.compile()
    return nc


# ---------------------------------------------------------------------------
# Entry point
# ---------------------------------------------------------------------------

def _make_inmaps(inputs, meta, idx16, drel, with_b):
    S, SPAD, ncores = meta["S"], meta["SPAD"], meta["ncores"]
    n_nodes = meta["n_nodes"]
    x = np.ascontiguousarray(np.asarray(inputs["x"], np.float32))
    w1 = np.asarray(inputs["W1"], np.float32)
    w2 = np.asarray(inputs["W2"], np.float32)
    no1 = np.asarray(inputs["noise1"], np.float32)
    no2 = np.asarray(inputs["noise2"], np.float32)

    def shard(arr, c):
        lo = c * S
        hi = min(lo + S, n_nodes)
        out = np.zeros((SPAD, 128), np.float32)
        out[:hi - lo] = arr[lo:hi]
        return out

    iota = np.tile(np.arange(128, dtype=np.float32), (128, 1))
    ident = np.eye(128, dtype=np.float32)
    in_maps = []
    for c in range(ncores):
        im = dict(
            xs=shard(x, c), n1s=shard(no1, c), n2s=shard(no2, c),
            w1t=np.ascontiguousarray(w1.T), w2t=np.ascontiguousarray(w2.T),
            idx=idx16[c], dstrel=drel[c], iota=iota, ident=ident,
        )
        if with_b:
            im["b1r"] = np.asarray(inputs["b1"], np.float32).reshape(1, 128)
            im["b2r"] = np.asarray(inputs["b2"], np.float32).reshape(1, 128)
        in_maps.append(im)
    return in_maps


def _run(inputs, ncores=NCORES, sim=False, trace=False):
    ei = np.asarray(inputs["edge_index"], np.int64)
    n_nodes = int(np.asarray(inputs["x"]).shape[0])
    meta, idx16, drel = _preprocess(ei[0], ei[1], n_nodes, ncores)
    with_b = bool(np.any(np.asarray(inputs["b1"])) or
                  np.any(np.asarray(inputs["b2"])))
    nc = _build_program(meta, with_b)
    in_maps = _make_inmaps(inputs, meta, idx16, drel, with_b)
    S, SPAD = meta["S"], meta["SPAD"]

    if sim:
        from concourse.bass_interp import MultiCoreSim
        msim = MultiCoreSim(nc, ncores, trace=trace)
        for c in range(ncores):
            for k, v in in_maps[c].items():
                msim.cores[c].tensor(k)[:] = v
        msim.simulate()
        results = [{"out": np.array(msim.cores[c].tensor("out"))}
                   for c in range(ncores)]
        res = msim
    else:
        res = run_bass_kernel_spmd(nc, in_maps, core_ids=list(range(ncores)),
                                   trace=trace)
        results = res.results

    parts = []
    for c in range(ncores):
        lo = c * S
        hi = min(lo + S, n_nodes)
        parts.append(results[c]["out"][:hi - lo])
    out = np.concatenate(parts, axis=0).astype(np.float32)
    return out, res


def kernel(**inputs) -> np.ndarray:
    out, _ = _run(inputs, ncores=NCORES, sim=False)
    return out
